# revision 1
# baseline (speedup 1.0000x reference)
"""LightGCN-style 3-layer graph propagation on 8 Trainium2 NeuronCores.

Computes, for the fixed-size problem (100K users, 200K items, D=64, 2M edges,
3 layers):
    x = concat(user_emb, item_emb)
    acc = x
    repeat 3x:  x = segment_sum(x[edge_dst] * edge_vals, edge_src); acc += x
    return acc/4 split into (users, items)

Strategy (1D edge parallelism, sharded by src-block):
  * The 128-node blocks of the padded node space are sorted by edge count and
    dealt snake-wise to the 8 cores, so slot j has (nearly) the same padded
    chunk count on every core -> one SPMD program, per-core differences live
    entirely in input data.
  * Host bins+sorts each core's edges by owning (src) block into 128-edge
    chunks; device gathers x[dst] rows via indirect DMA (128 rows/call - the
    HW honors exactly one offset per partition), scales by val (DVE), builds
    a 128x128 one-hot of the src offset via is_equal vs an iota (DVE), and
    accumulates onehot.T @ msgs in PSUM over each block's chunks (PE).
  * Block epilogue: SBUF f32 accumulator += psum; x_next block staged and
    written to DRAM; per layer an AllGather replicates x_next to all cores.
  * Each core returns its own acc/4 slice; host inverts the permutation.
"""

import math
import numpy as np

import concourse.bass as bass
import concourse.bacc as bacc
import concourse.mybir as mybir
import concourse.tile as tile
from concourse.bass import IndirectOffsetOnAxis
from concourse import bass_utils

DT = mybir.dt
BLK = 128

NU, NI, D, E, L, M = 100000, 200000, 64, 2000000, 3, 8
N = NU + NI
_NB0 = math.ceil(N / BLK)
NB_G = math.ceil(_NB0 / M) * M
N_PAD = NB_G * BLK
NBC = NB_G // M
NOWN = NBC * BLK
G = 32          # chunks per gather tile group
GP = 8          # chunks per one-hot build instruction
SG = 16         # slots per x_next staging flush
F32 = DT.float32


def _host_prep(user_emb, item_emb, edge_vals, edge_src, edge_dst):
    x0 = np.concatenate([np.asarray(user_emb), np.asarray(item_emb)], axis=0)
    x0 = np.ascontiguousarray(x0, dtype=np.float32)
    src = np.asarray(edge_src).astype(np.int64)
    dst = np.asarray(edge_dst).astype(np.int64)
    val = np.asarray(edge_vals).astype(np.float32)

    b = src // BLK
    cnt = np.bincount(b, minlength=NB_G)
    order = np.argsort(-cnt, kind="stable")

    blocks_of = np.empty((M, NBC), dtype=np.int64)
    cj = np.empty(NBC, dtype=np.int64)
    for j in range(NBC):
        octile = order[M * j: M * j + M]
        cj[j] = max(1, math.ceil(cnt[octile].max() / BLK))
        blocks_of[:, j] = octile[::-1] if (j % 2) else octile
    C = int(cj.sum())

    node_pos = np.empty(N_PAD, dtype=np.int64)
    ar = np.arange(BLK)
    for c in range(M):
        for j in range(NBC):
            B = blocks_of[c, j]
            node_pos[B * BLK:(B + 1) * BLK] = c * NOWN + j * BLK + ar

    starts = np.zeros(NB_G + 1, dtype=np.int64)
    starts[1:] = np.cumsum(cnt)
    eorder = np.argsort(b, kind="stable")
    slot_off = np.zeros(NBC + 1, dtype=np.int64)
    slot_off[1:] = np.cumsum(cj)

    gidx = node_pos[dst]
    EC = C * BLK
    idx_a = np.zeros((M, EC), dtype=np.int64)
    val_a = np.zeros((M, EC), dtype=np.float32)
    lsr_a = np.zeros((M, EC), dtype=np.float32)
    for c in range(M):
        for j in range(NBC):
            B = blocks_of[c, j]
            es = eorder[starts[B]:starts[B + 1]]
            base = slot_off[j] * BLK
            n = len(es)
            idx_a[c, base:base + n] = gidx[es]
            val_a[c, base:base + n] = val[es]
            lsr_a[c, base:base + n] = src[es] - B * BLK

    # ascending gather addresses within each chunk (one-hot fixes attribution)
    for c in range(M):
        ia = idx_a[c].reshape(C, BLK)
        so = np.argsort(ia, axis=1, kind="stable")
        idx_a[c] = np.take_along_axis(ia, so, axis=1).reshape(EC)
        val_a[c] = np.take_along_axis(val_a[c].reshape(C, BLK), so, axis=1).reshape(EC)
        lsr_a[c] = np.take_along_axis(lsr_a[c].reshape(C, BLK), so, axis=1).reshape(EC)

    def t128(a, dt_np):
        return np.ascontiguousarray(a.reshape(C, BLK).T.astype(dt_np))

    x0p = np.zeros((N_PAD, D), dtype=np.float32)
    x0p[node_pos[:N]] = x0
    iota = np.tile(np.arange(BLK, dtype=np.float32), (BLK, GP))

    in_maps = []
    for c in range(M):
        acc0 = np.ascontiguousarray(
            x0p[c * NOWN:(c + 1) * NOWN].reshape(NBC, BLK, D)
            .transpose(1, 0, 2).reshape(BLK, NBC * D))
        in_maps.append({
            "idxT": t128(idx_a[c], np.int32),
            "valT": t128(val_a[c], np.float32),
            "lsrcT": t128(lsr_a[c], np.float32),
            "iota": iota,
            "acc0": acc0,
            "x0f": x0p,
        })
    return in_maps, node_pos, cj


def _build_program(cj):
    C = int(np.sum(cj))
    nc = bacc.Bacc("TRN2", target_bir_lowering=False, debug=False,
                   num_devices=M)
    idxT = nc.dram_tensor("idxT", [BLK, C], DT.int32, kind="ExternalInput")
    valT = nc.dram_tensor("valT", [BLK, C], F32, kind="ExternalInput")
    lsrcT = nc.dram_tensor("lsrcT", [BLK, C], F32, kind="ExternalInput")
    iota = nc.dram_tensor("iota", [BLK, GP * BLK], F32, kind="ExternalInput")
    acc0 = nc.dram_tensor("acc0", [BLK, NBC * D], F32, kind="ExternalInput")
    x0f = nc.dram_tensor("x0f", [N_PAD, D], F32, kind="ExternalInput")
    out_own = nc.dram_tensor("out_own", [NOWN, D], F32, kind="ExternalOutput")

    sched = []
    for j in range(NBC):
        for q in range(int(cj[j])):
            sched.append((j, q, int(cj[j])))

    with tile.TileContext(nc) as tc:
        with (
            tc.tile_pool(name="const", bufs=1) as const,
            tc.tile_pool(name="sb", bufs=3) as sb,
            tc.tile_pool(name="pp", bufs=4, space="PSUM") as pp,
            tc.tile_pool(name="dram", bufs=1, space="DRAM") as dram,
        ):
            idx_s = const.tile([BLK, C], DT.int32)
            nc.sync.dma_start(out=idx_s[:], in_=idxT[:])
            val_s = const.tile([BLK, C], F32)
            nc.sync.dma_start(out=val_s[:], in_=valT[:])
            lsrc_s = const.tile([BLK, C], F32)
            nc.sync.dma_start(out=lsrc_s[:], in_=lsrcT[:])
            iota_s = const.tile([BLK, GP * BLK], F32)
            nc.sync.dma_start(out=iota_s[:], in_=iota[:])
            acc_s = const.tile([BLK, NBC * D], F32)
            nc.sync.dma_start(out=acc_s[:], in_=acc0[:])

            xg = [dram.tile([N_PAD, D], F32, addr_space="Shared",
                            name=f"xg{l}") for l in range(L - 1)]
            xloc = [dram.tile([NOWN, D], F32, name=f"xloc{l}")
                    for l in range(L - 1)]

            for layer in range(L):
                src_t = x0f if layer == 0 else xg[layer - 1]
                gt = pt = xst = psum_cur = None
                g0 = p0 = js0 = -1
                for t in range(C):
                    j, q, cjj = sched[t]
                    if t % G == 0:
                        g0 = t
                        nG = min(G, C - g0)
                        gt = sb.tile([BLK, G * D], F32, tag="gt")
                        gt3 = gt[:, :nG * D].rearrange("p (g d) -> p g d", d=D)
                        for u in range(nG):
                            nc.gpsimd.indirect_dma_start(
                                out=gt[:, u * D:(u + 1) * D],
                                out_offset=None,
                                in_=src_t[:],
                                in_offset=IndirectOffsetOnAxis(
                                    ap=idx_s[:, g0 + u:g0 + u + 1], axis=0),
                            )
                        nc.vector.tensor_tensor(
                            out=gt3, in0=gt3,
                            in1=val_s[:, g0:g0 + nG].to_broadcast([BLK, nG, D]),
                            op=mybir.AluOpType.mult,
                        )
                    if t % GP == 0:
                        p0 = t
                        nP = min(GP, C - p0)
                        pt = sb.tile([BLK, GP * BLK], F32, tag="pt")
                        nc.vector.tensor_tensor(
                            out=pt[:, :nP * BLK].rearrange(
                                "p (g n) -> p g n", n=BLK),
                            in0=lsrc_s[:, p0:p0 + nP].to_broadcast(
                                [BLK, nP, BLK]),
                            in1=iota_s[:, :nP * BLK].rearrange(
                                "p (g n) -> p g n", n=BLK),
                            op=mybir.AluOpType.is_equal,
                        )
                    if q == 0:
                        psum_cur = pp.tile([BLK, D], F32, tag="ps")
                    nc.tensor.matmul(
                        out=psum_cur[:],
                        lhsT=pt[:, (t - p0) * BLK:(t - p0 + 1) * BLK],
                        rhs=gt[:, (t - g0) * D:(t - g0 + 1) * D],
                        start=(q == 0),
                        stop=(q == cjj - 1),
                    )
                    if q == cjj - 1:
                        nc.vector.tensor_add(
                            out=acc_s[:, j * D:(j + 1) * D],
                            in0=acc_s[:, j * D:(j + 1) * D],
                            in1=psum_cur[:],
                        )
                        if layer < L - 1:
                            if j % SG == 0:
                                js0 = j
                                xst = sb.tile([BLK, SG * D], F32, tag="xst")
                            nc.scalar.copy(
                                out=xst[:, (j - js0) * D:(j - js0 + 1) * D],
                                in_=psum_cur[:])
                            if j == js0 + SG - 1 or j == NBC - 1:
                                nw = j - js0 + 1
                                xv = xloc[layer][:].rearrange(
                                    "(b p) d -> p b d", p=BLK)[:, js0:j + 1, :]
                                nc.sync.dma_start(
                                    out=xv,
                                    in_=xst[:, :nw * D].rearrange(
                                        "p (b d) -> p b d", d=D))
                if layer < L - 1:
                    nc.gpsimd.collective_compute(
                        "AllGather",
                        mybir.AluOpType.bypass,
                        replica_groups=[list(range(M))],
                        ins=[xloc[layer][:]],
                        outs=[xg[layer][:]],
                    )

            nc.vector.tensor_scalar_mul(acc_s[:], acc_s[:], 1.0 / (L + 1))
            nc.sync.dma_start(
                out=out_own[:].rearrange("(b p) d -> p b d", p=BLK),
                in_=acc_s[:].rearrange("p (b d) -> p b d", d=D))

    nc.compile()
    return nc


_PROGRAM_CACHE = {}


def kernel(user_emb, item_emb, edge_vals, edge_src, edge_dst):
    in_maps, node_pos, cj = _host_prep(
        user_emb, item_emb, edge_vals, edge_src, edge_dst)
    key = cj.tobytes()
    nc = _PROGRAM_CACHE.get(key)
    if nc is None:
        nc = _build_program(cj)
        _PROGRAM_CACHE[key] = nc
    res = bass_utils.run_bass_kernel_spmd(
        nc, in_maps, core_ids=list(range(M)), trace=False)
    big = np.concatenate([res.results[c]["out_own"] for c in range(M)], axis=0)
    out = big[node_pos[:N]]
    return out[:NU].astype(np.float32), out[NU:].astype(np.float32)
